# revision 60
# baseline (speedup 1.0000x reference)
"""Trainium2 Bass kernel for nn_Net_74552042324489.

Data-parallel over batch n=8 across 8 NeuronCores (1 sample/core).
v2: restructured for PE pipelining.
  - 448-col chunking everywhere (3136 = 7*448) -> uniform matmuls, no tail.
  - deep3/x2/features in bf16 (half DMA bytes, 1 cyc/col matmuls);
    _4 and cam stay fp32 (argmax-suppression is tie-sensitive).
  - PE warm-up burst at t=0 flips the HAM clock gate (1.2->2.4 GHz)
    while input DMAs stream.
  - Attention software-pipelined: per k-chunk, pout(b-1) is emitted after
    S(b) so exp(b-1) on ScalarE hides under the S matmul stream.
  - camT5 rows are [ones|bg|fg] so the softmax denominator lands on
    partition 0 (epilogue needs no row-move DMA).
"""

import os
import sys

sys.path.insert(0, "/opt/trn_rl_repo")

from contextlib import ExitStack

import numpy as np

import concourse.bass as bass
import concourse.tile as tile
from concourse import bacc, mybir
from concourse.bass_utils import run_bass_kernel_spmd
from concourse.masks import make_identity

F32 = mybir.dt.float32
BF16 = mybir.dt.bfloat16
AF = mybir.ActivationFunctionType
ALU = mybir.AluOpType

HW = 3136  # 56*56
NCH = 448  # column chunk: 3136 = 7 * 448
N_CORES = 8

_CACHE = {}


def _resize_mat(h_in: int, h_out: int) -> np.ndarray:
    ys = np.linspace(0.0, h_in - 1.0, h_out).astype(np.float32)
    y0 = np.floor(ys).astype(np.int64)
    y1 = np.minimum(y0 + 1, h_in - 1)
    w = (ys - y0).astype(np.float32)
    R = np.zeros((h_in, h_out), np.float32)
    for i in range(h_out):
        R[y0[i], i] += 1.0 - w[i]
        R[y1[i], i] += w[i]
    return R


def _resize_coeffs_112() -> tuple[np.ndarray, np.ndarray]:
    ys = np.linspace(0.0, 111.0, 56).astype(np.float32)
    y0 = np.floor(ys).astype(np.int64)
    w = (ys - y0).astype(np.float32)
    # structural property (verified): y0[i] == 2i for i < 55; y0[55] == 111
    return (1.0 - w).astype(np.float32), w.astype(np.float32)


def _build_program():
    nc = bacc.Bacc(
        "TRN2", target_bir_lowering=False, debug=False, num_devices=N_CORES
    )

    d_x4 = nc.dram_tensor("x4", [512, HW], F32, kind="ExternalInput")
    d_deep3 = nc.dram_tensor("deep3", [320, HW], BF16, kind="ExternalInput")
    d_x2 = nc.dram_tensor("x2", [128, 112 * 112], BF16, kind="ExternalInput")
    d_x = nc.dram_tensor("x", [3, 448, 448], BF16, kind="ExternalInput")
    d_fc8T = nc.dram_tensor("fc8T", [512, 4], F32, kind="ExternalInput")
    d_f83T = nc.dram_tensor("f83T", [128, 64], BF16, kind="ExternalInput")
    d_f84T = nc.dram_tensor("f84T", [320, 128], BF16, kind="ExternalInput")
    d_qkA = nc.dram_tensor("qkA", [128, 384], BF16, kind="ExternalInput")
    d_qkB = nc.dram_tensor("qkB", [67, 384], BF16, kind="ExternalInput")
    d_a112 = nc.dram_tensor("a112", [128, 56], F32, kind="ExternalInput")
    d_b112 = nc.dram_tensor("b112", [128, 56], F32, kind="ExternalInput")
    d_rh = nc.dram_tensor("rh448", [448, 56], BF16, kind="ExternalInput")
    d_rw = nc.dram_tensor("rw448", [448, 56], BF16, kind="ExternalInput")
    d_out = nc.dram_tensor("out", [4, HW], F32, kind="ExternalOutput")

    EPS = 1e-05
    CHN = [(i * NCH, NCH) for i in range(7)]
    HBLK = [(i * 128, 128) for i in range(24)] + [(3072, 64)]

    with tile.TileContext(nc) as tc, ExitStack() as top:
        wpool = top.enter_context(tc.tile_pool(name="wpool", bufs=1))
        persist = top.enter_context(tc.tile_pool(name="persist", bufs=1))
        small = top.enter_context(tc.tile_pool(name="small", bufs=2))

        # ---- PE warm-up: junk matmuls flip the HAM clock gate to 8/8
        # while the first input DMAs are in flight.  Two PSUM banks
        # alternate so consecutive matmuls pipeline (high duty cycle).
        junkW = wpool.tile([128, 128], BF16, tag="junkW")
        nc.gpsimd.memset(junkW[:], 0.0)
        junkR = wpool.tile([128, 512], BF16, tag="junkR")
        nc.gpsimd.memset(junkR[:], 0.0)
        def warm_burst(n):
            with tc.tile_pool(name="warm", bufs=2,
                              space=bass.MemorySpace.PSUM) as warmp:
                for _ in range(n):
                    jp = warmp.tile([128, 512], F32, tag="junkP")
                    nc.tensor.matmul(jp[:], junkW[:], junkR[:], start=True,
                                     stop=True)

        warm_burst(20)

        # ---- first-needed input DMAs, then weights ----
        x4r = d_x4.ap().rearrange("(k p) n -> p k n", p=128)
        d3r = d_deep3.ap()[0:256, :].rearrange("(k p) n -> p k n", p=128)
        cfstack = ExitStack()
        cfs = cfstack.enter_context(tc.tile_pool(name="cfs", bufs=3))
        st4_0 = cfs.tile([128, 4, NCH], F32, tag="x4st")
        nc.sync.dma_start(st4_0[:], x4r[:, :, 0:NCH])
        d3a_0 = cfs.tile([128, 2, NCH], BF16, tag="d3a")
        nc.sync.dma_start(d3a_0[:], d3r[:, :, 0:NCH])
        d3b_0 = cfs.tile([64, NCH], BF16, tag="d3b")
        nc.sync.dma_start(d3b_0[:], d_deep3.ap()[256:320, 0:NCH])
        fc8T = wpool.tile([128, 4, 4], F32, tag="fc8T")
        nc.sync.dma_start(fc8T[:], d_fc8T.ap().rearrange("(k p) o -> p k o", p=128))
        f84T_01 = wpool.tile([128, 2, 128], BF16, tag="f84T01")
        nc.sync.dma_start(
            f84T_01[:], d_f84T.ap()[0:256, :].rearrange("(k p) o -> p k o", p=128)
        )
        f84T_2 = wpool.tile([64, 128], BF16, tag="f84T2")
        nc.sync.dma_start(f84T_2[:], d_f84T.ap()[256:320, :])
        a112 = wpool.tile([128, 56], F32, tag="a112")
        nc.sync.dma_start(a112[:], d_a112.ap())
        b112 = wpool.tile([128, 56], F32, tag="b112")
        nc.sync.dma_start(b112[:], d_b112.ap())
        x2stack = ExitStack()
        x2pool = x2stack.enter_context(tc.tile_pool(name="x2pool", bufs=1))
        x2full = x2pool.tile([128, 112 * 112], BF16, tag="x2full")
        nc.sync.dma_start(x2full[:, 0:6272], d_x2.ap()[:, 0:6272])
        nc.sync.dma_start(x2full[:, 6272:12544], d_x2.ap()[:, 6272:12544])
        xsb = wpool.tile([112, 4, 3, 448], BF16, tag="xsb")
        xdr = d_x.ap().rearrange("c h w -> h c w")
        for hc in range(4):
            nc.sync.dma_start(xsb[:, hc], xdr[112 * hc:112 * (hc + 1), :, :])
        f83T = wpool.tile([128, 64], BF16, tag="f83T")
        nc.sync.dma_start(f83T[:], d_f83T.ap())
        rh = wpool.tile([112, 4, 56], BF16, tag="rh")
        nc.sync.dma_start(rh[:], d_rh.ap().rearrange("(k p) o -> p k o", p=112))
        rw = wpool.tile([112, 4, 56], BF16, tag="rw")
        nc.sync.dma_start(rw[:], d_rw.ap().rearrange("(k p) o -> p k o", p=112))
        qkA = wpool.tile([128, 384], BF16, tag="qkA")
        nc.sync.dma_start(qkA[:], d_qkA.ap())
        qkB = wpool.tile([67, 384], BF16, tag="qkB")
        nc.sync.dma_start(qkB[:], d_qkB.ap())
        ident = wpool.tile([128, 128], F32, tag="ident")
        make_identity(nc, ident[:])
        idb = wpool.tile([128, 128], BF16, tag="idb")
        nc.vector.tensor_copy(idb[:], ident[:])

        # ---- persistent intermediates ----
        # 25 blocks x 33 cols: [bg|fg1..3|zeros...|ones@32] -- the ones col
        # lands the softmax denominator on PSUM partition 32 (aligned), so
        # the epilogue avoids the row-move DMA.
        camT5 = persist.tile([128, 25 * 33], BF16, tag="camT5")
        nc.vector.memset(camT5[:], 0.0)
        f_a = persist.tile([128, HW], BF16, tag="f_a")  # = f8_4
        f_b = persist.tile([67, HW], BF16, tag="f_b")  # = [f8_3(64); x_s(3)]
        qA = persist.tile([128, HW], BF16, tag="qA")
        qB = persist.tile([128, HW], BF16, tag="qB")  # qB duplicated (rows 64:128)
        kA = persist.tile([128, HW], BF16, tag="kA")
        kB = persist.tile([128, HW], BF16, tag="kB")  # kB duplicated (rows 64:128)
        cam = persist.tile([4, HW], F32, tag="cam")
        out_sb = persist.tile([4, HW], F32, tag="out_sb")

        # ================= x2 -> x2r (stride-2 bilinear, DVE) =================
        x2v = x2full[:].rearrange("p (h w) -> p h w", h=112)
        x2w = x2pool.tile([128, 112 * 56], BF16, tag="x2w")
        x2wv = x2w[:].rearrange("p (h w) -> p h w", h=112)
        x2r = persist.tile([128, HW], BF16, tag="x2r")
        x2rv = x2r[:].rearrange("p (h w) -> p h w", h=56)
        HC = 14
        with tc.tile_pool(name="rsz", bufs=2) as rszp:
            for hc in range(112 // HC):
                eng = nc.vector
                tg = "v"
                sv = x2v[:, hc * HC:(hc + 1) * HC, :]
                dst = x2wv[:, hc * HC:(hc + 1) * HC, :]
                even = sv[:, :, 0:110:2]
                odd = sv[:, :, 1:111:2]
                abc = a112[:, 0:55].unsqueeze(1).broadcast_to([128, HC, 55])
                bbc = b112[:, 0:55].unsqueeze(1).broadcast_to([128, HC, 55])
                t1 = rszp.tile([128, HC, 55], F32, tag="t1" + tg)
                eng.tensor_tensor(t1[:], even, abc, op=ALU.mult)
                t2 = rszp.tile([128, HC, 55], F32, tag="t2" + tg)
                eng.tensor_tensor(t2[:], odd, bbc, op=ALU.mult)
                eng.tensor_tensor(dst[:, :, 0:55], t1[:], t2[:], op=ALU.add)
                eng.tensor_copy(dst[:, :, 55:56], sv[:, :, 111:112])
            for jc, jl in ((0, 28), (28, 27)):
                everow = x2wv[:, 2 * jc:2 * (jc + jl) - 1:2, :]
                oddrow = x2wv[:, 2 * jc + 1:2 * (jc + jl):2, :]
                arow = a112[:, jc:jc + jl].unsqueeze(2).broadcast_to([128, jl, 56])
                brow = b112[:, jc:jc + jl].unsqueeze(2).broadcast_to([128, jl, 56])
                t3 = rszp.tile([128, 28, 56], F32, tag="t1v")
                nc.vector.tensor_tensor(t3[:, 0:jl, :], everow, arow, op=ALU.mult)
                t4 = rszp.tile([128, 28, 56], F32, tag="t2v")
                nc.vector.tensor_tensor(t4[:, 0:jl, :], oddrow, brow, op=ALU.mult)
                nc.vector.tensor_tensor(
                    x2rv[:, jc:jc + jl, :], t3[:, 0:jl, :], t4[:, 0:jl, :],
                    op=ALU.add,
                )
            nc.vector.tensor_copy(x2rv[:, 55:56, :], x2wv[:, 111:112, :])
        x2stack.close()

        # ================= cam + f8_4 per 448-col chunk =================
        # camT transposes of the raw cam interleave with the convs (keeps
        # PE dense); per-chunk min/max partials keep the lane-starved
        # [4, .] reductions off the critical path.
        camTall = persist.tile([128, 25, 4], F32, tag="camTall")
        nc.gpsimd.memset(camTall[64:128, 24, :], 0.0)
        mnp = small.tile([4, 7], F32, tag="mnp", bufs=1)
        mxp = small.tile([4, 7], F32, tag="mxp", bufs=1)
        # blocks fully covered after chunk ci (448*(ci+1) >= 128*(b+1))
        TGRP = [(0, 3), (3, 7), (7, 10), (10, 14), (14, 17), (17, 21), (21, 25)]
        with tc.tile_pool(name="cfp", bufs=2, space=bass.MemorySpace.PSUM) as cfp, \
             tc.tile_pool(name="ffp", bufs=2, space=bass.MemorySpace.PSUM) as ffp, \
             tc.tile_pool(name="ctp", bufs=2, space=bass.MemorySpace.PSUM) as ctp:
            for ci, (no, nl) in enumerate(CHN):
                if ci == 0:
                    st4, d3a, d3b = st4_0, d3a_0, d3b_0
                else:
                    st4 = cfs.tile([128, 4, NCH], F32, tag="x4st")
                    nc.sync.dma_start(st4[:], x4r[:, :, no:no + nl])
                    d3a = cfs.tile([128, 2, NCH], BF16, tag="d3a")
                    nc.sync.dma_start(d3a[:], d3r[:, :, no:no + nl])
                    d3b = cfs.tile([64, NCH], BF16, tag="d3b")
                    nc.sync.dma_start(d3b[:], d_deep3.ap()[256:320, no:no + nl])
                cp = cfp.tile([4, NCH], F32, tag="campsum")
                for ck in range(4):
                    nc.tensor.matmul(
                        cp[:], fc8T[:, ck, :], st4[:, ck, :],
                        start=(ck == 0), stop=(ck == 3),
                    )
                fp = ffp.tile([128, NCH], F32, tag="f4psum")
                nc.tensor.matmul(
                    fp[:], f84T_01[:, 0, :], d3a[:, 0, :], start=True, stop=False
                )
                nc.tensor.matmul(
                    fp[:], f84T_01[:, 1, :], d3a[:, 1, :], start=False, stop=False
                )
                nc.tensor.matmul(
                    fp[:], f84T_2[:], d3b[:], start=False, stop=True
                )
                nc.scalar.copy(cam[:, no:no + nl], cp[:])
                nc.scalar.activation(f_a[:, no:no + nl], fp[:], AF.Relu)
                nc.vector.tensor_reduce(
                    mnp[:, ci:ci + 1], cam[:, no:no + nl],
                    axis=mybir.AxisListType.X, op=ALU.min,
                )
                nc.vector.tensor_reduce(
                    mxp[:, ci:ci + 1], cam[:, no:no + nl],
                    axis=mybir.AxisListType.X, op=ALU.max,
                )
                for bi in range(*TGRP[ci]):
                    ho, hl = HBLK[bi]
                    tpc = ctp.tile([128, 4], F32, tag="tpsum")
                    nc.tensor.transpose(
                        tpc[0:hl, :], cam[:, ho:ho + hl], ident[0:4, 0:4]
                    )
                    nc.vector.tensor_copy(camTall[0:hl, bi, :], tpc[0:hl, :])

        # ================= x -> x_s -> f_b[64:67] =================
        with tc.tile_pool(name="p4s", bufs=2) as p4s, \
             tc.tile_pool(name="p4sb", bufs=1) as p4sb, \
             tc.tile_pool(name="p4p", bufs=1, space=bass.MemorySpace.PSUM) as p4p:
            xh = p4sb.tile([56, 3, 448], BF16, tag="xh")
            for c in range(3):
                xp = p4p.tile([56, 448], F32, tag="xhp", bufs=2)
                for hc in range(4):
                    nc.tensor.matmul(
                        xp[:], rh[:, hc, :], xsb[:, hc, c, :],
                        start=(hc == 0), stop=(hc == 3),
                    )
                nc.vector.tensor_copy(xh[:, c, :], xp[:])
            xhT = p4sb.tile([112, 12, 56], BF16, tag="xhT")
            for c in range(3):
                for wc in range(4):
                    tp = p4p.tile([112, 56], BF16, tag="xtp", bufs=2)
                    nc.tensor.transpose(
                        tp[:], xh[:, c, 112 * wc:112 * (wc + 1)], idb[0:56, 0:56]
                    )
                    nc.vector.tensor_copy(xhT[:, c * 4 + wc, :], tp[:])
            xs3 = p4sb.tile([3, HW], BF16, tag="xs3")
            for c in range(3):
                wpz = p4p.tile([56, 56], F32, tag="xwp", bufs=2)
                for wc in range(4):
                    nc.tensor.matmul(
                        wpz[:], xhT[:, c * 4 + wc, :], rw[:, wc, :],
                        start=(wc == 0), stop=(wc == 3),
                    )
                ws = p4s.tile([56, 56], BF16, tag="xws")
                nc.vector.tensor_copy(ws[:], wpz[:])
                nc.sync.dma_start(xs3[c:c + 1, :], ws[:])
            nc.vector.tensor_copy(f_b[64:67, :], xs3[:])

        cfstack.close()

        # ---- combine min/max partials; normalize camTall in transposed
        # space (128 lanes) instead of the lane-starved [4, 3136] layout.
        mn = small.tile([4, 1], F32, tag="mn")
        mx = small.tile([4, 1], F32, tag="mx")
        nc.vector.tensor_reduce(mn[:], mnp[:], axis=mybir.AxisListType.X, op=ALU.min)
        nc.vector.tensor_reduce(mx[:], mxp[:], axis=mybir.AxisListType.X, op=ALU.max)
        rng = small.tile([4, 1], F32, tag="rng")
        nc.vector.tensor_tensor(rng[:], mx[:], mn[:], op=ALU.subtract)
        nc.vector.tensor_scalar_add(rng[:], rng[:], EPS)
        rs = small.tile([4, 1], F32, tag="rs")
        nc.vector.reciprocal(rs[:], rng[:])
        mn4 = small.tile([1, 4], F32, tag="mn4")
        nc.sync.dma_start(mn4[:], mn[:])
        rs4 = small.tile([1, 4], F32, tag="rs4")
        nc.sync.dma_start(rs4[:], rs[:])
        mnb = small.tile([128, 4], F32, tag="mnb")
        nc.gpsimd.partition_broadcast(mnb[:], mn4[:])
        rsb = small.tile([128, 4], F32, tag="rsb")
        nc.gpsimd.partition_broadcast(rsb[:], rs4[:])
        camN = persist.tile([128, 25, 4], F32, tag="camN")
        nc.vector.tensor_tensor(
            camN[:], camTall[:], mnb[:].unsqueeze(1).broadcast_to([128, 25, 4]),
            op=ALU.subtract,
        )
        nc.vector.tensor_tensor(
            camN[:], camN[:], rsb[:].unsqueeze(1).broadcast_to([128, 25, 4]),
            op=ALU.mult,
        )

        # ---- camT5 = [bg | suppressed fg | ... | ones] per block ----
        c5v = camT5[:].rearrange("p (b f) -> p b f", f=33)
        nc.vector.memset(c5v[:, :, 32], 1.0)
        fm = small.tile([128, 25], F32, tag="fm")
        nc.vector.tensor_reduce(
            fm[:], camN[:, :, 1:4], axis=mybir.AxisListType.X, op=ALU.max
        )
        nc.vector.tensor_scalar(
            c5v[:, :, 0], fm[:], -1.0, 1.0, op0=ALU.mult, op1=ALU.add
        )
        msk = small.tile([128, 25, 3], F32, tag="msk")
        fmb = fm[:].unsqueeze(2).broadcast_to([128, 25, 3])
        nc.vector.tensor_tensor(msk[:], camN[:, :, 1:4], fmb, op=ALU.is_ge)
        nc.vector.tensor_tensor(
            c5v[:, :, 1:4], camN[:, :, 1:4], msk[:], op=ALU.mult
        )

        # ================= f8_3 =================
        with tc.tile_pool(name="p3p", bufs=2, space=bass.MemorySpace.PSUM) as p3p:
            for no, nl in CHN:
                fp3 = p3p.tile([64, NCH], F32, tag="f3psum")
                nc.tensor.matmul(
                    fp3[:], f83T[:], x2r[:, no:no + nl], start=True, stop=True
                )
                nc.scalar.activation(f_b[0:64, no:no + nl], fp3[:], AF.Relu)

        # ================= q,k =================
        # qB/kB land in both partition halves so paired h-blocks can run
        # their 64-deep S2 matmuls in disjoint PE row-groups concurrently.
        MCH = [(qA, 0, 128), (qB, 128, 64), (kA, 192, 128), (kB, 320, 64)]
        with tc.tile_pool(name="p6p", bufs=3, space=bass.MemorySpace.PSUM) as p6p:
            for no, nl in CHN:
                for dst, mo, ml in MCH:
                    qp = p6p.tile([128, NCH], F32, tag="qkpsum")
                    nc.tensor.matmul(
                        qp[0:ml, :], qkA[:, mo:mo + ml], f_a[:, no:no + nl],
                        start=True, stop=False,
                    )
                    nc.tensor.matmul(
                        qp[0:ml, :], qkB[:, mo:mo + ml], f_b[:, no:no + nl],
                        start=False, stop=True,
                    )
                    nc.scalar.copy(dst[0:ml, no:no + nl], qp[0:ml, :])
                    if ml == 64:
                        nc.scalar.copy(dst[64:128, no:no + nl], qp[0:ml, :])

        # re-warm the clock gate before the long attention stream
        warm_burst(12)

        # ================= attention =================
        with tc.tile_pool(name="p7e", bufs=4) as p7e, \
             tc.tile_pool(name="p7r", bufs=2) as p7r, \
             tc.tile_pool(name="p7s", bufs=4, space=bass.MemorySpace.PSUM) as p7s, \
             tc.tile_pool(name="p7o", bufs=2, space=bass.MemorySpace.PSUM) as p7o:
            for ko, kl in CHN:
                pout = p7o.tile([33, NCH], F32, tag="pout")

                def emit_pout(pend, pout=pout):
                    pb, phl, pet = pend
                    nc.tensor.matmul(
                        pout[:], camT5[0:phl, pb * 33:pb * 33 + 33], pet[0:phl, :],
                        start=(pb == 0), stop=(pb == 24),
                    )

                pending = []
                bi = 0
                while bi < 25:
                    pair = [bi, bi + 1] if bi + 1 < 25 else [bi]
                    sps = []
                    for b in pair:
                        ho, hl = HBLK[b]
                        sp = p7s.tile([128, NCH], F32, tag="spsum")
                        nc.tensor.matmul(
                            sp[0:hl, :], qA[:, ho:ho + hl], kA[:, ko:ko + kl],
                            start=True, stop=False,
                        )
                        sps.append(sp)
                    for j, b in enumerate(pair):
                        ho, hl = HBLK[b]
                        nc.tensor.matmul(
                            sps[j][0:hl, :], qB[64 * j:64 * (j + 1), ho:ho + hl],
                            kB[64 * j:64 * (j + 1), ko:ko + kl],
                            start=False, stop=True,
                            tile_position=(64 * j, 0),
                        )
                    for j, b in enumerate(pair):
                        ho, hl = HBLK[b]
                        et = p7e.tile([128, NCH], BF16, tag="exptile")
                        nc.scalar.activation(et[0:hl, :], sps[j][0:hl, :], AF.Exp)
                        pending.append((b, hl, et))
                    while len(pending) > 2:
                        emit_pout(pending.pop(0))
                    bi += len(pair)
                for pend in pending:
                    emit_pout(pend)
                # epilogue: rows 0:4 / denominator (aligned psum partition 32)
                ot5 = p7r.tile([4, NCH], F32, tag="ot5")
                nc.vector.tensor_copy(ot5[:], pout[0:4, :])
                den = p7r.tile([1, NCH], F32, tag="den")
                nc.vector.tensor_copy(den[:], pout[32:33, :])
                rcp = p7r.tile([1, NCH], F32, tag="rcp")
                rsc = p7r.tile([1, NCH], F32, tag="rsc")
                nc.vector.reciprocal_approx_accurate(rcp[:], den[:], rsc[:])
                rb = p7r.tile([4, NCH], F32, tag="rb")
                nc.gpsimd.partition_broadcast(rb[:], rcp[0:1, :])
                nc.gpsimd.tensor_tensor(
                    out_sb[:, ko:ko + kl], ot5[:], rb[:], op=ALU.mult
                )
                nc.sync.dma_start(d_out.ap()[:, ko:ko + kl], out_sb[:, ko:ko + kl])

    nc.compile()
    return nc


def _get_program():
    if "nc" not in _CACHE:
        _CACHE["nc"] = _build_program()
    return _CACHE["nc"]


def _host_prep(inputs: dict) -> list[dict]:
    import ml_dtypes

    BFNP = ml_dtypes.bfloat16
    x = np.asarray(inputs["x"], np.float32).astype(BFNP)
    x2 = np.asarray(inputs["x2"], np.float32).astype(BFNP)
    deep3 = np.asarray(inputs["deep3"], np.float32).astype(BFNP)
    _4 = np.ascontiguousarray(np.asarray(inputs["_4"], np.float32))
    fc8_w = np.asarray(inputs["fc8_w"], np.float32)
    f83_w = np.asarray(inputs["f83_w"], np.float32)
    f84_w = np.asarray(inputs["f84_w"], np.float32)
    f91_w = np.asarray(inputs["f91_w"], np.float32)
    f92_w = np.asarray(inputs["f92_w"], np.float32)

    n = x.shape[0]
    fc8T = np.ascontiguousarray(fc8_w.T)                    # [512, 4] f32
    f83T = np.ascontiguousarray(f83_w.T).astype(BFNP)       # [128, 64]
    f84T = np.ascontiguousarray(f84_w.T).astype(BFNP)       # [320, 128]
    # f channel permutation: [f8_4 (128), f8_3 (64), x_s (3)]
    perm = np.concatenate([np.arange(67, 195), np.arange(3, 67), np.arange(3)])
    wqk = np.concatenate([f91_w, f92_w], axis=0)[:, perm]   # [384, 195]
    wqkT = np.ascontiguousarray(wqk.T)                      # [195, 384]
    qkA = np.ascontiguousarray(wqkT[0:128]).astype(BFNP)
    qkB = np.ascontiguousarray(wqkT[128:195]).astype(BFNP)
    a112, b112 = _resize_coeffs_112()
    a112 = np.ascontiguousarray(np.broadcast_to(a112, (128, 56)))
    b112 = np.ascontiguousarray(np.broadcast_to(b112, (128, 56)))
    rh448 = _resize_mat(448, 56).astype(BFNP)

    shared = {
        "fc8T": fc8T, "f83T": f83T, "f84T": f84T, "qkA": qkA, "qkB": qkB,
        "a112": a112, "b112": b112, "rh448": rh448, "rw448": rh448,
    }
    in_maps = []
    for i in range(n):
        m = dict(shared)
        m["x4"] = _4[i].reshape(512, HW)
        m["deep3"] = np.ascontiguousarray(deep3[i].reshape(320, HW))
        m["x2"] = np.ascontiguousarray(x2[i].reshape(128, 112 * 112))
        m["x"] = np.ascontiguousarray(x[i])
        in_maps.append(m)
    return in_maps


def _install_ntff_hook() -> bool:
    """Register the NTFF profile hook that the agent image's antenv lacks."""
    try:
        import types

        import antenv

        if "antenv.axon_hooks" not in sys.modules:
            mod = types.ModuleType("antenv.axon_hooks")
            store = {"h": None}
            mod.set_axon_ntff_profile_hook = lambda h: store.update(h=h)
            mod.get_axon_ntff_profile_hook = lambda: store["h"]
            sys.modules["antenv.axon_hooks"] = mod
            antenv.axon_hooks = mod
            from trn_agent_boot.trn_boot import _ntff_profile_via_ctypes

            hook = _ntff_profile_via_ctypes("/opt/axon/libaxon_pjrt.so")
            if hook is None:
                return False
            mod.set_axon_ntff_profile_hook(hook)
        return sys.modules["antenv.axon_hooks"].get_axon_ntff_profile_hook() is not None
    except Exception as e:  # profiling is best-effort
        print(f"ntff hook install failed: {e}", file=sys.stderr)
        return False


def kernel(**inputs) -> np.ndarray:
    nc = _get_program()
    in_maps = _host_prep(inputs)
    trace = bool(int(os.environ.get("KERNEL_PROFILE", "0")))
    if trace:
        trace = _install_ntff_hook()
    res = run_bass_kernel_spmd(nc, in_maps, core_ids=list(range(N_CORES)),
                               trace=trace)
    _CACHE["last_result"] = res
    out = np.stack([r["out"] for r in res.results]).reshape(8, 4, 56, 56)
    return out.astype(np.float32)
